# revision 3
# baseline (speedup 1.0000x reference)
"""Expert-parallel MoE SwiGLU kernel for 8 Trainium2 NeuronCores.

Strategy: expert parallelism with host-side dispatch/combine. Each of the
8 cores owns one expert's weights. The host routes tokens by expert_idx,
packs each expert's tokens as a transposed [D, W] panel (features on
partitions so no on-chip transposes are needed anywhere), and each core
runs a dense SwiGLU FFN:  yT = w_down.T-blocks @ (silu(wg.T@xT) * (wu.T@xT)).
Matmul operands stream as fp16 (fp32 PSUM accumulation; ~6e-4 max
relative error vs the fp32 reference), halving the weight traffic that
dominates this memory-bound kernel.

Schedule (v2): f is processed in 8 half-groups (hg) of 512 columns.
hg0 runs d-OUTER: its 8 PSUM accumulation chains (4 gate + 4 up) are
fed one d-chunk at a time, so real matmuls start as soon as the first
[128,512] weight tile lands instead of waiting for the whole first
group — this replaces most of the HAM warm-up dummy burst and the
DMA-gated gaps of the old f-major opening. hg1..7 run f-major with the
previous half-group's down-projection chains interleaved between f-tile
slots (8 chains of 4 matmuls per hg), which leaves only hg7's 32 down
matmuls after the last gate/up matmul. Weight DMA issue is split across
both HWDGE rings (sync + scalar). y write-out is per-d-chunk, issued as
soon as each chunk's final accumulation lands, alternating rings.
"""

import numpy as np
from contextlib import ExitStack

D_MODEL = 1024
D_FF = 4096
N_EXPERTS = 8
N_CORES = 8

_ND = D_MODEL // 128   # 8 contraction chunks over d_model
_NHG = 8               # half-groups over f
_HGW = D_FF // _NHG    # 512 f columns per half-group
_FTG = _HGW // 128     # 4 f-tiles per half-group

_nc_cache = {}

import os as _os
_CDT = _os.environ.get("MOE_KERNEL_DTYPE", "float16")


def _np_cdt():
    if _CDT == "float16":
        return np.float16
    if _CDT == "bfloat16":
        import ml_dtypes
        return ml_dtypes.bfloat16
    return np.float32


def _build_nc(W: int):
    """Build + schedule the per-core Bass program for token capacity W."""
    import concourse.bacc as bacc
    import concourse.tile as tile
    from concourse import mybir

    f32 = mybir.dt.float32
    f32r = getattr(mybir.dt, _CDT)

    nc = bacc.Bacc("TRN2", target_bir_lowering=False, debug=False,
                   num_devices=N_CORES)
    xt = nc.dram_tensor("xt", [D_MODEL, W], f32r, kind="ExternalInput").ap()
    wg = nc.dram_tensor("wg", [_NHG, _ND, 128, _HGW], f32r,
                        kind="ExternalInput").ap()
    wu = nc.dram_tensor("wu", [_NHG, _ND, 128, _HGW], f32r,
                        kind="ExternalInput").ap()
    wd = nc.dram_tensor("wd", [D_FF, D_MODEL], f32r, kind="ExternalInput").ap()
    yt = nc.dram_tensor("yt", [D_MODEL, W], f32, kind="ExternalOutput").ap()

    _ctr = [0]

    def _nm():
        _ctr[0] += 1
        return _ctr[0]

    with tile.TileContext(nc) as tc, ExitStack() as ctx:
        xpool = ctx.enter_context(tc.tile_pool(name="x", bufs=1))
        wgp = ctx.enter_context(tc.tile_pool(name="wgp", bufs=3))
        wup = ctx.enter_context(tc.tile_pool(name="wup", bufs=3))
        wdp = ctx.enter_context(tc.tile_pool(name="wdp", bufs=2))
        tp = ctx.enter_context(tc.tile_pool(name="tp", bufs=2))
        gap = ctx.enter_context(tc.tile_pool(name="gap", bufs=3))
        yp = ctx.enter_context(tc.tile_pool(name="yp", bufs=1))
        pp = ctx.enter_context(tc.tile_pool(name="pp", bufs=8, space="PSUM"))

        # Input activations, transposed: d_model on partitions. Split
        # across both HWDGE rings so the first tiles land early.
        xts = []
        for d in range(_ND):
            x_t = xpool.tile([128, W], f32r, tag=f"x{d}")
            eng = nc.sync if d % 2 == 0 else nc.scalar
            eng.dma_start(x_t[:], xt[d * 128:(d + 1) * 128, :])
            xts.append(x_t)

        y_acc = [yp.tile([128, W], f32, tag=f"y{d}", name=f"y_acc{d}")
                 for d in range(_ND)]

        # Small HAM pre-trip burst: a few full-width dummy matmuls run
        # while the x tiles / first weight tile are still in flight, so
        # the PE activity window starts filling ~1us earlier.
        scr_w = xpool.tile([128, 128], f32r, tag="scrw", name="scr_w")
        scr_x = xpool.tile([128, W], f32r, tag="scrx", name="scr_x")
        nc.vector.memset(scr_w[:], 0.0)
        nc.vector.memset(scr_x[:], 0.0)
        scr_p = [pp.tile([128, W], f32, tag="ps", name=f"scr_p{i}")
                 for i in range(2)]
        for i in range(3):
            nc.tensor.matmul(scr_p[i % 2][:], scr_w[:], scr_x[:],
                             start=True, stop=True)

        # -------- DMA issue helpers (alternate rings) --------
        def fetch_gu(h, d):
            g_t = wgp.tile([128, _HGW], f32r, tag=f"wg{d}")
            nc.sync.dma_start(g_t[:], wg[h, d])
            u_t = wup.tile([128, _HGW], f32r, tag=f"wu{d}")
            nc.scalar.dma_start(u_t[:], wu[h, d])
            return g_t, u_t

        def fetch_wd(h):
            tiles = []
            for ft in range(_FTG):
                fc = h * _FTG + ft
                wd_t = wdp.tile([128, D_MODEL], f32r, tag=f"wd{ft}")
                eng = nc.sync if ft % 2 == 0 else nc.scalar
                eng.dma_start(wd_t[:], wd[fc * 128:(fc + 1) * 128, :])
                tiles.append(wd_t)
            return tiles

        # -------- down-projection: one dt chain over a half-group -----
        def emit_down_chain(dt, t_tiles, wd_tiles, first_hg, last=False):
            pdt = pp.tile([128, W], f32, tag="ps", name=f"pd_{_nm()}")
            for ft in range(_FTG):
                nc.tensor.matmul(
                    pdt[:],
                    wd_tiles[ft][:, dt * 128:(dt + 1) * 128],
                    t_tiles[ft][:],
                    start=(ft == 0), stop=(ft == _FTG - 1))
            if first_hg:
                nc.vector.tensor_copy(y_acc[dt][:], pdt[:])
            else:
                nc.vector.tensor_add(y_acc[dt][:], y_acc[dt][:], pdt[:])
            if last:
                eng = nc.sync if dt % 2 == 0 else nc.scalar
                eng.dma_start(yt[dt * 128:(dt + 1) * 128, :], y_acc[dt][:])

        # -------- gate/up + SwiGLU for one f-tile (f-major form) ------
        def emit_gu_tile(gu, ft):
            g_t, u_t = gu
            psg = pp.tile([128, W], f32, tag="ps", name=f"pg_{_nm()}")
            for d in range(_ND):
                nc.tensor.matmul(
                    psg[:], g_t[d][:, ft * 128:(ft + 1) * 128], xts[d][:],
                    start=(d == 0), stop=(d == _ND - 1))
            psu = pp.tile([128, W], f32, tag="ps", name=f"pu_{_nm()}")
            for d in range(_ND):
                nc.tensor.matmul(
                    psu[:], u_t[d][:, ft * 128:(ft + 1) * 128], xts[d][:],
                    start=(d == 0), stop=(d == _ND - 1))
            return _swiglu(psg, psu, ft)

        def _swiglu(psg, psu, ft):
            g_act = gap.tile([128, W], f32, tag="gact")
            nc.scalar.activation(g_act[:], psg[:],
                                 mybir.ActivationFunctionType.Silu)
            t_t = tp.tile([128, W], f32r, tag=f"t{ft}")
            nc.vector.tensor_mul(t_t[:], g_act[:], psu[:])
            return t_t

        # ---------------- half-group 0: d-outer opening ----------------
        gu0 = ([None] * _ND, [None] * _ND)
        for d in range(_ND):
            gu0[0][d], gu0[1][d] = fetch_gu(0, d)
        wd_prev = fetch_wd(0)

        psg0 = [pp.tile([128, W], f32, tag="ps", name=f"pg0_{i}") for i in range(_FTG)]
        psu0 = [pp.tile([128, W], f32, tag="ps", name=f"pu0_{i}") for i in range(_FTG)]
        for d in range(_ND):
            for ft in range(_FTG):
                nc.tensor.matmul(
                    psg0[ft][:],
                    gu0[0][d][:, ft * 128:(ft + 1) * 128], xts[d][:],
                    start=(d == 0), stop=(d == _ND - 1))
                nc.tensor.matmul(
                    psu0[ft][:],
                    gu0[1][d][:, ft * 128:(ft + 1) * 128], xts[d][:],
                    start=(d == 0), stop=(d == _ND - 1))
        t_prev = [_swiglu(psg0[ft], psu0[ft], ft) for ft in range(_FTG)]

        # ---------------- half-groups 1..7: f-major + downs ------------
        for h in range(1, _NHG):
            gu = ([None] * _ND, [None] * _ND)
            for d in range(_ND):
                gu[0][d], gu[1][d] = fetch_gu(h, d)
            wd_cur = fetch_wd(h)

            t_cur = []
            for ft in range(_FTG):
                t_cur.append(emit_gu_tile(gu, ft))
                # interleave 2 down chains of the previous half-group
                emit_down_chain(2 * ft, t_prev, wd_prev, first_hg=(h == 1))
                emit_down_chain(2 * ft + 1, t_prev, wd_prev,
                                first_hg=(h == 1))
            t_prev, wd_prev = t_cur, wd_cur

        # ---------------- tail: downs of the last half-group -----------
        for dt in range(_ND):
            emit_down_chain(dt, t_prev, wd_prev, first_hg=False, last=True)

    nc.compile()
    return nc


def _pack_gu(w):
    # [D, F] -> [NHG, ND, 128, HGW] so each streamed tile is contiguous
    w = np.asarray(w).astype(_np_cdt())
    return np.ascontiguousarray(
        w.reshape(_ND, 128, _NHG, _HGW).transpose(2, 0, 1, 3))


def _run_one(W, tok_lists, x_flat, packed_w, out_flat):
    from concourse.bass_utils import run_bass_kernel_spmd

    if W not in _nc_cache:
        _nc_cache[W] = _build_nc(W)
    nc = _nc_cache[W]

    D = x_flat.shape[1]
    in_maps = []
    for e in range(N_EXPERTS):
        toks = tok_lists[e]
        xt_e = np.zeros((D, W), dtype=_np_cdt())
        xt_e[:, :len(toks)] = x_flat[toks].T.astype(_np_cdt())
        in_maps.append({
            "xt": xt_e,
            "wg": packed_w[e][0],
            "wu": packed_w[e][1],
            "wd": packed_w[e][2],
        })

    res = None
    for attempt in range(3):
        try:
            res = run_bass_kernel_spmd(nc, in_maps,
                                       core_ids=list(range(N_CORES)))
            break
        except Exception:
            if attempt == 2:
                raise
            import time
            time.sleep(3.0)
            try:
                import jax
                jax.clear_caches()
                jax.clear_backends()
            except Exception:
                pass
    for e in range(N_EXPERTS):
        toks = tok_lists[e]
        out_flat[toks] = res.results[e]["yt"][:, :len(toks)].T


def kernel(x, expert_idx, w_gate, w_up, w_down):
    x = np.asarray(x, dtype=np.float32)
    idx = np.asarray(expert_idx).astype(np.int64)
    B, S, D = x.shape
    T = B * S
    x_flat = np.ascontiguousarray(x.reshape(T, D))
    idx_flat = idx.reshape(T)

    packed_w = [
        (_pack_gu(w_gate[e]), _pack_gu(w_up[e]),
         np.ascontiguousarray(np.asarray(w_down[e]).astype(_np_cdt())))
        for e in range(N_EXPERTS)
    ]

    tok_lists = [np.nonzero(idx_flat == e)[0] for e in range(N_EXPERTS)]
    cap = max(1, max(len(t) for t in tok_lists))
    out_flat = np.zeros((T, D), dtype=np.float32)

    if cap <= 512:
        # normal path: one SPMD run, capacity = max expert load (floor 256
        # keeps DMA partition lines >= 512B)
        W = max(256, cap)
        _run_one(W, tok_lists, x_flat, packed_w, out_flat)
    else:
        # fallback for extreme routing imbalance: process tokens in
        # rounds of <=512 per expert, reusing one compiled W=512 program
        rounds = -(-cap // 512)
        for r in range(rounds):
            round_lists = [t[r * 512:(r + 1) * 512] for t in tok_lists]
            _run_one(512, round_lists, x_flat, packed_w, out_flat)

    return out_flat.reshape(B, S, D)


# revision 5
# speedup vs baseline: 1.0070x; 1.0070x over previous
"""Expert-parallel MoE SwiGLU kernel for 8 Trainium2 NeuronCores.

Strategy: expert parallelism with host-side dispatch/combine. Each of the
8 cores owns one expert's weights. The host routes tokens by expert_idx,
packs each expert's tokens as a transposed [D, W] panel (features on
partitions so no on-chip transposes are needed anywhere), and each core
runs a dense SwiGLU FFN:  yT = w_down.T-blocks @ (silu(wg.T@xT) * (wu.T@xT)).
Matmul operands stream as fp16 (fp32 PSUM accumulation; ~6e-4 max
relative error vs the fp32 reference), halving the weight traffic that
dominates this memory-bound kernel.

Schedule (v3): baseline f-major pipeline over 4 f-groups of 1024 with
the previous group's down-projection interleaved, plus three opening
fixes that remove the long HAM-warmup dummy burst and the DMA-gated
gaps of the original opening:
 - x is shipped as ONE [128, 8W] DMA (host packs d-chunks side by side)
   so the sync ring spends 0.65us on it instead of 5us of issue time.
 - group 0's first 512 f-columns run d-OUTER: 4 gate + 4 up PSUM chains
   are fed one d-chunk at a time, so real matmuls start as soon as the
   first [128,1024] weight tile lands (~9us) instead of after all 8
   gate tiles (~12us). wu group-0 tiles stream on the scalar ring,
   which is idle before the first activations.
 - only ~6 warmup dummies (to pre-trip the HAM activity window during
   the unavoidable DMA lead-in) instead of 16.
Tail: final y write-outs alternate sync/scalar rings.
"""

import numpy as np
from contextlib import ExitStack

D_MODEL = 1024
D_FF = 4096
N_EXPERTS = 8
N_CORES = 8

_ND = D_MODEL // 128  # 8 contraction chunks over d_model
_NF = D_FF // 128     # 32 f chunks

_nc_cache = {}

import os as _os
_CDT = _os.environ.get("MOE_KERNEL_DTYPE", "float16")

_FSG = 1024           # f columns per gate/up weight streaming tile
_NFSG = D_FF // _FSG  # 4 groups
_FTG = _FSG // 128    # 8 f-tiles per group


def _np_cdt():
    if _CDT == "float16":
        return np.float16
    if _CDT == "bfloat16":
        import ml_dtypes
        return ml_dtypes.bfloat16
    return np.float32


def _build_nc(W: int):
    """Build + schedule the per-core Bass program for token capacity W."""
    import concourse.bacc as bacc
    import concourse.tile as tile
    from concourse import mybir

    f32 = mybir.dt.float32
    f32r = getattr(mybir.dt, _CDT)

    nc = bacc.Bacc("TRN2", target_bir_lowering=False, debug=False,
                   num_devices=N_CORES)
    xt = nc.dram_tensor("xt", [128, _ND * W], f32r, kind="ExternalInput").ap()
    wg = nc.dram_tensor("wg", [_NFSG, _ND, 128, _FSG], f32r,
                        kind="ExternalInput").ap()
    wu = nc.dram_tensor("wu", [_NFSG, _ND, 128, _FSG], f32r,
                        kind="ExternalInput").ap()
    wd = nc.dram_tensor("wd", [D_FF, D_MODEL], f32r, kind="ExternalInput").ap()
    yt = nc.dram_tensor("yt", [D_MODEL, W], f32, kind="ExternalOutput").ap()

    _ctr = [0]

    def _nm():
        _ctr[0] += 1
        return _ctr[0]

    with tile.TileContext(nc) as tc, ExitStack() as ctx:
        xpool = ctx.enter_context(tc.tile_pool(name="x", bufs=1))
        wgp = ctx.enter_context(tc.tile_pool(name="wgp", bufs=4))
        wup = ctx.enter_context(tc.tile_pool(name="wup", bufs=4))
        wdp = ctx.enter_context(tc.tile_pool(name="wdp", bufs=3))
        tp = ctx.enter_context(tc.tile_pool(name="tp", bufs=2))
        gap = ctx.enter_context(tc.tile_pool(name="gap", bufs=3))
        yp = ctx.enter_context(tc.tile_pool(name="yp", bufs=1))
        pp = ctx.enter_context(tc.tile_pool(name="pp", bufs=8, space="PSUM"))

        # All input activations in ONE DMA: host packs xT d-chunks side
        # by side as [128, 8W]; xts[d] are column slices.
        x_all = xpool.tile([128, _ND * W], f32r, tag="xall", name="x_all")
        nc.sync.dma_start(x_all[:], xt[:, :])
        xts = [x_all[:, d * W:(d + 1) * W] for d in range(_ND)]

        y_acc = [yp.tile([128, W], f32, tag=f"y{d}", name=f"y_acc{d}")
                 for d in range(_ND)]

        # Warmup scratch: dummy matmuls pre-trip the HAM activity window
        # while x / the first weight tiles are still in flight.
        scr_w = xpool.tile([128, 128], f32r, tag="scrw", name="scr_w")
        scr_x = xpool.tile([128, W], f32r, tag="scrx", name="scr_x")
        nc.vector.memset(scr_w[:], 0.0)
        nc.vector.memset(scr_x[:], 0.0)
        scr_p = [pp.tile([128, W], f32, tag="ps", name=f"scr_p{i}")
                 for i in range(2)]
        for i in range(6):
            nc.tensor.matmul(scr_p[i % 2][:], scr_w[:], scr_x[:],
                             start=True, stop=True)

        # Group 0 weight tiles: wg on sync ring, wu on the (idle) scalar
        # ring, in d order so the d-outer opening consumes them as they
        # land.
        wg0, wu0 = [], []
        for d in range(_ND):
            g_t = wgp.tile([128, _FSG], f32r, tag=f"wg{d}", name=f"wg0_{d}")
            nc.sync.dma_start(g_t[:], wg[0, d])
            wg0.append(g_t)
            u_t = wup.tile([128, _FSG], f32r, tag=f"wu{d}", name=f"wu0_{d}")
            nc.scalar.dma_start(u_t[:], wu[0, d])
            wu0.append(u_t)

        def _swiglu(psg, psu, ft):
            g_act = gap.tile([128, W], f32, tag="gact", name=f"ga_{_nm()}")
            nc.scalar.activation(g_act[:], psg[:],
                                 mybir.ActivationFunctionType.Silu)
            t_t = tp.tile([128, W], f32r, tag=f"t{ft}", name=f"t_{_nm()}")
            nc.vector.tensor_mul(t_t[:], g_act[:], psu[:])
            return t_t

        # Down-projection of the PREVIOUS f group is interleaved between
        # this group's matmul bursts.
        def emit_down(fsg, t_tiles, wd_tiles, dts, last=False):
            for dt in dts:
                pdt = pp.tile([128, W], f32, tag="ps", name=f"pd_{_nm()}")
                for ft in range(_FTG):
                    nc.tensor.matmul(
                        pdt[:],
                        wd_tiles[ft][:, dt * 128:(dt + 1) * 128],
                        t_tiles[ft][:],
                        start=(ft == 0), stop=(ft == _FTG - 1))
                if fsg == 0:
                    nc.vector.tensor_copy(y_acc[dt][:], pdt[:])
                else:
                    nc.vector.tensor_add(y_acc[dt][:], y_acc[dt][:], pdt[:])
                if last:
                    eng = nc.sync if dt % 2 == 0 else nc.scalar
                    eng.dma_start(yt[dt * 128:(dt + 1) * 128, :],
                                  y_acc[dt][:])

        # ---------------- group 0 ----------------
        # Phase A: f-tiles 0..3 d-OUTER — chains fill as weight tiles
        # arrive; PE is busy from the first tile instead of waiting for
        # all eight.
        psg0 = [pp.tile([128, W], f32, tag="ps", name=f"pg0_{i}")
                for i in range(4)]
        psu0 = [pp.tile([128, W], f32, tag="ps", name=f"pu0_{i}")
                for i in range(4)]
        for d in range(_ND):
            for ft in range(4):
                nc.tensor.matmul(
                    psg0[ft][:], wg0[d][:, ft * 128:(ft + 1) * 128],
                    xts[d], start=(d == 0), stop=(d == _ND - 1))
                nc.tensor.matmul(
                    psu0[ft][:], wu0[d][:, ft * 128:(ft + 1) * 128],
                    xts[d], start=(d == 0), stop=(d == _ND - 1))
        t_g0 = [_swiglu(psg0[ft], psu0[ft], ft) for ft in range(4)]

        # wd tiles for group 0 (consumed by downs during group 1)
        wd_prev = []
        for ft in range(_FTG):
            wd_t = wdp.tile([128, D_MODEL], f32r, tag=f"wd{ft}",
                            name=f"wd0_{ft}")
            nc.sync.dma_start(wd_t[:], wd[ft * 128:(ft + 1) * 128, :])
            wd_prev.append(wd_t)

        # Phase B: f-tiles 4..7 f-major (all tiles resident by now)
        for ft in range(4, _FTG):
            psg = pp.tile([128, W], f32, tag="ps", name=f"pg_{_nm()}")
            for d in range(_ND):
                nc.tensor.matmul(
                    psg[:], wg0[d][:, ft * 128:(ft + 1) * 128], xts[d],
                    start=(d == 0), stop=(d == _ND - 1))
            psu = pp.tile([128, W], f32, tag="ps", name=f"pu_{_nm()}")
            for d in range(_ND):
                nc.tensor.matmul(
                    psu[:], wu0[d][:, ft * 128:(ft + 1) * 128], xts[d],
                    start=(d == 0), stop=(d == _ND - 1))
            t_g0.append(_swiglu(psg, psu, ft))

        prev = (0, t_g0, wd_prev)

        # ---------------- groups 1..3 ----------------
        for fsg in range(1, _NFSG):
            wg_t, wu_t = [], []
            for d in range(_ND):
                g_t = wgp.tile([128, _FSG], f32r, tag=f"wg{d}",
                               name=f"wg{fsg}_{d}")
                nc.sync.dma_start(g_t[:], wg[fsg, d])
                wg_t.append(g_t)
                u_t = wup.tile([128, _FSG], f32r, tag=f"wu{d}",
                               name=f"wu{fsg}_{d}")
                nc.sync.dma_start(u_t[:], wu[fsg, d])
                wu_t.append(u_t)

            t_tiles = []
            wd_tiles = []
            for ft in range(_FTG):
                fc = fsg * _FTG + ft
                wd_t = wdp.tile([128, D_MODEL], f32r, tag=f"wd{ft}",
                                name=f"wd{fsg}_{ft}")
                nc.sync.dma_start(wd_t[:], wd[fc * 128:(fc + 1) * 128, :])
                wd_tiles.append(wd_t)
                psg = pp.tile([128, W], f32, tag="ps", name=f"pg_{_nm()}")
                for d in range(_ND):
                    nc.tensor.matmul(
                        psg[:], wg_t[d][:, ft * 128:(ft + 1) * 128],
                        xts[d], start=(d == 0), stop=(d == _ND - 1))
                psu = pp.tile([128, W], f32, tag="ps", name=f"pu_{_nm()}")
                for d in range(_ND):
                    nc.tensor.matmul(
                        psu[:], wu_t[d][:, ft * 128:(ft + 1) * 128],
                        xts[d], start=(d == 0), stop=(d == _ND - 1))
                t_tiles.append(_swiglu(psg, psu, ft))
                emit_down(prev[0], prev[1], prev[2], (ft,))
            prev = (fsg, t_tiles, wd_tiles)

        # ---------------- tail: downs of the last group ----------------
        emit_down(prev[0], prev[1], prev[2], range(_ND), last=True)

    nc.compile()
    return nc


def _pack_gu(w):
    # [D, F] -> [NFSG, ND, 128, FSG] so each streamed tile is contiguous
    w = np.asarray(w).astype(_np_cdt())
    return np.ascontiguousarray(
        w.reshape(_ND, 128, _NFSG, _FSG).transpose(2, 0, 1, 3))


def _run_one(W, tok_lists, x_flat, packed_w, out_flat):
    from concourse.bass_utils import run_bass_kernel_spmd

    if W not in _nc_cache:
        _nc_cache[W] = _build_nc(W)
    nc = _nc_cache[W]

    D = x_flat.shape[1]
    in_maps = []
    for e in range(N_EXPERTS):
        toks = tok_lists[e]
        # xT packed as [128, ND*W]: d-chunk d occupies cols [d*W,(d+1)*W)
        xt_e = np.zeros((128, _ND * W), dtype=_np_cdt())
        xe = x_flat[toks].T.astype(_np_cdt())          # [D, n]
        n = len(toks)
        for d in range(_ND):
            xt_e[:, d * W:d * W + n] = xe[d * 128:(d + 1) * 128, :]
        in_maps.append({
            "xt": xt_e,
            "wg": packed_w[e][0],
            "wu": packed_w[e][1],
            "wd": packed_w[e][2],
        })

    res = None
    for attempt in range(3):
        try:
            res = run_bass_kernel_spmd(nc, in_maps,
                                       core_ids=list(range(N_CORES)))
            break
        except Exception:
            if attempt == 2:
                raise
            import time
            time.sleep(3.0)
            try:
                import jax
                jax.clear_caches()
                jax.clear_backends()
            except Exception:
                pass
    for e in range(N_EXPERTS):
        toks = tok_lists[e]
        out_flat[toks] = res.results[e]["yt"][:, :len(toks)].T


def kernel(x, expert_idx, w_gate, w_up, w_down):
    x = np.asarray(x, dtype=np.float32)
    idx = np.asarray(expert_idx).astype(np.int64)
    B, S, D = x.shape
    T = B * S
    x_flat = np.ascontiguousarray(x.reshape(T, D))
    idx_flat = idx.reshape(T)

    packed_w = [
        (_pack_gu(w_gate[e]), _pack_gu(w_up[e]),
         np.ascontiguousarray(np.asarray(w_down[e]).astype(_np_cdt())))
        for e in range(N_EXPERTS)
    ]

    tok_lists = [np.nonzero(idx_flat == e)[0] for e in range(N_EXPERTS)]
    cap = max(1, max(len(t) for t in tok_lists))
    out_flat = np.zeros((T, D), dtype=np.float32)

    if cap <= 512:
        # normal path: one SPMD run, capacity = max expert load (floor 256
        # keeps DMA partition lines >= 512B)
        W = max(256, cap)
        _run_one(W, tok_lists, x_flat, packed_w, out_flat)
    else:
        # fallback for extreme routing imbalance: process tokens in
        # rounds of <=512 per expert, reusing one compiled W=512 program
        rounds = -(-cap // 512)
        for r in range(rounds):
            round_lists = [t[r * 512:(r + 1) * 512] for t in tok_lists]
            _run_one(512, round_lists, x_flat, packed_w, out_flat)

    return out_flat.reshape(B, S, D)


# revision 6
# speedup vs baseline: 1.1693x; 1.1612x over previous
"""Expert-parallel MoE SwiGLU kernel for 8 Trainium2 NeuronCores.

Strategy: expert parallelism with host-side dispatch/combine. Each of the
8 cores owns one expert's weights. The host routes tokens by expert_idx,
packs each expert's tokens as a transposed [D, W] panel (features on
partitions so no on-chip transposes are needed anywhere), and each core
runs a dense SwiGLU FFN:  yT = w_down.T-blocks @ (silu(wg.T@xT) * (wu.T@xT)).
Matmul operands stream as fp16 (fp32 PSUM accumulation; ~6e-4 max
relative error vs the fp32 reference), halving the weight traffic that
dominates this memory-bound kernel. Set MOE_KERNEL_DTYPE=float32r for a
full-precision fp32 variant (~2.4e-4, ~1.5x slower).
"""

import numpy as np
from contextlib import ExitStack

D_MODEL = 1024
D_FF = 4096
N_EXPERTS = 8
N_CORES = 8

_ND = D_MODEL // 128  # 8 contraction chunks over d_model
_NF = D_FF // 128     # 32 f chunks

_nc_cache = {}

# compute dtype for matmul operands: "float32r" (safest), "float16", "bfloat16"
import os as _os
_CDT = _os.environ.get("MOE_KERNEL_DTYPE", "float16")

# f columns per gate/up weight streaming group: keep DMA lines at 2KB
_FSG = 512 if _CDT == "float32r" else 1024
_NFSG = D_FF // _FSG
_FTG = _FSG // 128    # f-tiles per group


def _np_cdt():
    if _CDT == "float16":
        return np.float16
    if _CDT == "bfloat16":
        import ml_dtypes
        return ml_dtypes.bfloat16
    return np.float32


def _build_nc(W: int):
    """Build + schedule the per-core Bass program for token capacity W."""
    import concourse.bacc as bacc
    import concourse.tile as tile
    from concourse import mybir

    f32 = mybir.dt.float32
    f32r = getattr(mybir.dt, _CDT)

    nc = bacc.Bacc("TRN2", target_bir_lowering=False, debug=False,
                   num_devices=N_CORES)
    xt = nc.dram_tensor("xt", [D_MODEL, W], f32r, kind="ExternalInput").ap()
    wg = nc.dram_tensor("wg", [_NFSG, _ND, 128, _FSG], f32r,
                        kind="ExternalInput").ap()
    wu = nc.dram_tensor("wu", [_NFSG, _ND, 128, _FSG], f32r,
                        kind="ExternalInput").ap()
    wd = nc.dram_tensor("wd", [D_FF, D_MODEL], f32r, kind="ExternalInput").ap()
    yt = nc.dram_tensor("yt", [D_MODEL, W], f32, kind="ExternalOutput").ap()

    with tile.TileContext(nc) as tc, ExitStack() as ctx:
        xpool = ctx.enter_context(tc.tile_pool(name="x", bufs=1))
        wgp = ctx.enter_context(tc.tile_pool(name="wgp", bufs=4))
        wup = ctx.enter_context(tc.tile_pool(name="wup", bufs=4))
        wdp = ctx.enter_context(tc.tile_pool(name="wdp", bufs=3))
        tp = ctx.enter_context(tc.tile_pool(name="tp", bufs=2))
        gap = ctx.enter_context(tc.tile_pool(name="gap", bufs=3))
        yp = ctx.enter_context(tc.tile_pool(name="yp", bufs=1))
        pg = ctx.enter_context(tc.tile_pool(name="pg", bufs=2, space="PSUM"))
        pu = ctx.enter_context(tc.tile_pool(name="pu", bufs=2, space="PSUM"))
        pd = ctx.enter_context(tc.tile_pool(name="pd", bufs=4, space="PSUM"))

        # Input activations, transposed: d_model on partitions.
        xts = []
        for d in range(_ND):
            x_t = xpool.tile([128, W], f32r, tag=f"x{d}")
            nc.scalar.dma_start(x_t[:], xt[d * 128:(d + 1) * 128, :])
            xts.append(x_t)

        y_acc = [yp.tile([128, W], f32, tag=f"y{d}", name=f"y_acc{d}")
                 for d in range(_ND)]

        # HAM warm-up scratch: dummy matmuls interleaved through f group 0
        # keep the PE activity monitor busy while real weights stream in,
        # so real matmuls run at 2.4GHz instead of the cold 1.2GHz.
        scr_w = xpool.tile([128, 128], f32r, tag="scrw", name="scr_w")
        scr_x = xpool.tile([128, W], f32r, tag="scrx", name="scr_x")
        nc.vector.memset(scr_w[:], 0.0)
        nc.vector.memset(scr_x[:], 0.0)
        scr_p = pd.tile([128, W], f32, tag="pd", name="scr_p")
        scr_p2 = pd.tile([128, W], f32, tag="pd", name="scr_p2")
        _scr = [scr_p, scr_p2]

        def emit_warmup(n):
            # full-width dummies: HAM counts streaming activity, so narrow
            # dummies under-feed the busy window (measured: 23us vs 10us
            # to warm)
            for i in range(n):
                nc.tensor.matmul(_scr[i % 2][:], scr_w[:], scr_x[:],
                                 start=True, stop=True)

        # dense opening burst: ~3.6us of continuous PE activity while the
        # first weight tiles are still in flight trips the HAM busy window
        # before the first real matmul, so everything runs at 2.4GHz.
        emit_warmup(16)

        # Fused pipeline over f groups: gate/up matmuls + SwiGLU produce
        # short-lived t tiles; the down-projection of the PREVIOUS f group
        # is interleaved between this group's matmul bursts so the PE's
        # DMA-wait gaps are broken into sub-HAM-window slices. Weight DMA
        # issue is split across both HWDGE rings (sync + scalar engines).
        def emit_down(fsg, t_tiles, wd_tiles, dts):
            # y[dt] += wd[fgroup rows, dt cols].T @ t   for dt in dts
            for dt in dts:
                pdt = pd.tile([128, W], f32, tag="pd", name=f"pd_{fsg}_{dt}")
                for ft in range(_FTG):
                    nc.tensor.matmul(
                        pdt[:],
                        wd_tiles[ft][:, dt * 128:(dt + 1) * 128],
                        t_tiles[ft][:],
                        start=(ft == 0), stop=(ft == _FTG - 1))
                if fsg == 0:
                    nc.vector.tensor_copy(y_acc[dt][:], pdt[:])
                else:
                    nc.vector.tensor_add(y_acc[dt][:], y_acc[dt][:], pdt[:])

        prev = None  # (fsg, t_tiles, wd_tiles) of the previous f group
        for fsg in range(_NFSG):
            wg_t, wu_t = [], []
            if fsg == 0:
                # gate tiles first: the first matmul group needs all 8
                for d in range(_ND):
                    g_t = wgp.tile([128, _FSG], f32r, tag=f"wg{d}")
                    nc.sync.dma_start(g_t[:], wg[fsg, d])
                    wg_t.append(g_t)
                for d in range(_ND):
                    u_t = wup.tile([128, _FSG], f32r, tag=f"wu{d}")
                    nc.sync.dma_start(u_t[:], wu[fsg, d])
                    wu_t.append(u_t)
            else:
                for d in range(_ND):
                    g_t = wgp.tile([128, _FSG], f32r, tag=f"wg{d}")
                    nc.sync.dma_start(g_t[:], wg[fsg, d])
                    wg_t.append(g_t)
                    u_t = wup.tile([128, _FSG], f32r, tag=f"wu{d}")
                    nc.sync.dma_start(u_t[:], wu[fsg, d])
                    wu_t.append(u_t)

            def g_slice(d, ft):
                return wg_t[d][:, ft * 128:(ft + 1) * 128]

            def u_slice(d, ft):
                return wu_t[d][:, ft * 128:(ft + 1) * 128]

            t_tiles = []
            wd_tiles = []
            for ft in range(_FTG):
                fc = fsg * _FTG + ft
                wd_t = wdp.tile([128, D_MODEL], f32r, tag=f"wd{ft}")
                nc.sync.dma_start(wd_t[:], wd[fc * 128:(fc + 1) * 128, :])
                wd_tiles.append(wd_t)
                if fsg == 0 and ft < 4:
                    emit_warmup(2)
                psg = pg.tile([128, W], f32)
                for d in range(_ND):
                    nc.tensor.matmul(
                        psg[:],
                        g_slice(d, ft),
                        xts[d][:],
                        start=(d == 0), stop=(d == _ND - 1))
                if fsg == 0 and ft < 4:
                    emit_warmup(2)
                psu = pu.tile([128, W], f32)
                for d in range(_ND):
                    nc.tensor.matmul(
                        psu[:],
                        u_slice(d, ft),
                        xts[d][:],
                        start=(d == 0), stop=(d == _ND - 1))
                g_act = gap.tile([128, W], f32, tag="gact")
                nc.scalar.activation(g_act[:], psg[:],
                                     mybir.ActivationFunctionType.Silu)
                t_t = tp.tile([128, W], f32r, tag=f"t{ft}")
                nc.vector.tensor_mul(t_t[:], g_act[:], psu[:])
                t_tiles.append(t_t)
                if prev is not None:
                    if _FTG == 8:
                        emit_down(prev[0], prev[1], prev[2], (ft,))
                    else:
                        emit_down(prev[0], prev[1], prev[2], (2 * ft, 2 * ft + 1))
            prev = (fsg, t_tiles, wd_tiles)
        emit_down(prev[0], prev[1], prev[2], range(_ND))

        for dt in range(_ND):
            nc.sync.dma_start(yt[dt * 128:(dt + 1) * 128, :], y_acc[dt][:])

    nc.compile()
    return nc


def _pack_gu(w):
    # [D, F] -> [NFSG, ND, 128, FSG] so each streamed tile is contiguous
    w = np.asarray(w).astype(_np_cdt())
    return np.ascontiguousarray(
        w.reshape(_ND, 128, _NFSG, _FSG).transpose(2, 0, 1, 3))


def _run_one(W, tok_lists, x_flat, packed_w, out_flat):
    from concourse.bass_utils import run_bass_kernel_spmd

    if W not in _nc_cache:
        _nc_cache[W] = _build_nc(W)
    nc = _nc_cache[W]

    D = x_flat.shape[1]
    in_maps = []
    for e in range(N_EXPERTS):
        toks = tok_lists[e]
        xt_e = np.zeros((D, W), dtype=_np_cdt())
        xt_e[:, :len(toks)] = x_flat[toks].T.astype(_np_cdt())
        in_maps.append({
            "xt": xt_e,
            "wg": packed_w[e][0],
            "wu": packed_w[e][1],
            "wd": packed_w[e][2],
        })

    res = None
    for attempt in range(3):
        try:
            res = run_bass_kernel_spmd(nc, in_maps,
                                       core_ids=list(range(N_CORES)))
            break
        except Exception:
            if attempt == 2:
                raise
            import time
            time.sleep(3.0)
            try:
                import jax
                jax.clear_caches()
                jax.clear_backends()
            except Exception:
                pass
    for e in range(N_EXPERTS):
        toks = tok_lists[e]
        out_flat[toks] = res.results[e]["yt"][:, :len(toks)].T


def kernel(x, expert_idx, w_gate, w_up, w_down):
    x = np.asarray(x, dtype=np.float32)
    idx = np.asarray(expert_idx).astype(np.int64)
    B, S, D = x.shape
    T = B * S
    x_flat = np.ascontiguousarray(x.reshape(T, D))
    idx_flat = idx.reshape(T)

    packed_w = [
        (_pack_gu(w_gate[e]), _pack_gu(w_up[e]),
         np.ascontiguousarray(np.asarray(w_down[e]).astype(_np_cdt())))
        for e in range(N_EXPERTS)
    ]

    tok_lists = [np.nonzero(idx_flat == e)[0] for e in range(N_EXPERTS)]
    cap = max(1, max(len(t) for t in tok_lists))
    out_flat = np.zeros((T, D), dtype=np.float32)

    if cap <= 512:
        # normal path: one SPMD run, capacity = max expert load (floor 256
        # keeps DMA partition lines >= 512B)
        W = max(256, cap)
        _run_one(W, tok_lists, x_flat, packed_w, out_flat)
    else:
        # fallback for extreme routing imbalance: process tokens in
        # rounds of <=512 per expert, reusing one compiled W=512 program
        rounds = -(-cap // 512)
        for r in range(rounds):
            round_lists = [t[r * 512:(r + 1) * 512] for t in tok_lists]
            _run_one(512, round_lists, x_flat, packed_w, out_flat)

    return out_flat.reshape(B, S, D)



# revision 8
# speedup vs baseline: 1.1844x; 1.0129x over previous
"""Expert-parallel MoE SwiGLU kernel for 8 Trainium2 NeuronCores.

Strategy: expert parallelism with host-side dispatch/combine. Each of the
8 cores owns one expert's weights. The host routes tokens by expert_idx,
packs each expert's tokens as a transposed [D, W] panel (features on
partitions so no on-chip transposes are needed anywhere), and each core
runs a dense SwiGLU FFN:  yT = w_down.T-blocks @ (silu(wg.T@xT) * (wu.T@xT)).
Matmul operands stream as fp16 (fp32 PSUM accumulation; ~6e-4 max
relative error vs the fp32 reference), halving the weight traffic that
dominates this memory-bound kernel.

Schedule (v3): baseline f-major pipeline over 4 f-groups of 1024 with
the previous group's down-projection interleaved, plus three opening
fixes that remove the long HAM-warmup dummy burst and the DMA-gated
gaps of the original opening:
 - x is shipped as ONE [128, 8W] DMA (host packs d-chunks side by side)
   so the sync ring spends 0.65us on it instead of 5us of issue time.
 - group 0's first 512 f-columns run d-OUTER: 4 gate + 4 up PSUM chains
   are fed one d-chunk at a time, so real matmuls start as soon as the
   first [128,1024] weight tile lands (~9us) instead of after all 8
   gate tiles (~12us). wu group-0 tiles stream on the scalar ring,
   which is idle before the first activations.
 - only ~6 warmup dummies (to pre-trip the HAM activity window during
   the unavoidable DMA lead-in) instead of 16.
Tail: final y write-outs alternate sync/scalar rings.
"""

import numpy as np
from contextlib import ExitStack

D_MODEL = 1024
D_FF = 4096
N_EXPERTS = 8
N_CORES = 8

_ND = D_MODEL // 128  # 8 contraction chunks over d_model
_NF = D_FF // 128     # 32 f chunks

_nc_cache = {}

import os as _os
_CDT = _os.environ.get("MOE_KERNEL_DTYPE", "float16")

_FSG = 1024           # f columns per gate/up weight streaming tile
_NFSG = D_FF // _FSG  # 4 groups
_FTG = _FSG // 128    # 8 f-tiles per group


def _np_cdt():
    if _CDT == "float16":
        return np.float16
    if _CDT == "bfloat16":
        import ml_dtypes
        return ml_dtypes.bfloat16
    return np.float32


def _build_nc(W: int):
    """Build + schedule the per-core Bass program for token capacity W."""
    import concourse.bacc as bacc
    import concourse.tile as tile
    from concourse import mybir

    f32 = mybir.dt.float32
    f32r = getattr(mybir.dt, _CDT)

    nc = bacc.Bacc("TRN2", target_bir_lowering=False, debug=False,
                   num_devices=N_CORES)
    Wp = (W + 31) // 32 * 32   # 64B-aligned d-chunk slots
    xt = nc.dram_tensor("xt", [128, _ND * Wp], f32r, kind="ExternalInput").ap()
    wg = nc.dram_tensor("wg", [_NFSG, _ND, 128, _FSG], f32r,
                        kind="ExternalInput").ap()
    wu = nc.dram_tensor("wu", [_NFSG, _ND, 128, _FSG], f32r,
                        kind="ExternalInput").ap()
    wd = nc.dram_tensor("wd", [D_FF, D_MODEL], f32r, kind="ExternalInput").ap()
    yt = nc.dram_tensor("yt", [D_MODEL, W], f32, kind="ExternalOutput").ap()

    _ctr = [0]

    def _nm():
        _ctr[0] += 1
        return _ctr[0]

    with tile.TileContext(nc) as tc, ExitStack() as ctx:
        xpool = ctx.enter_context(tc.tile_pool(name="x", bufs=1))
        wgp = ctx.enter_context(tc.tile_pool(name="wgp", bufs=4))
        wup = ctx.enter_context(tc.tile_pool(name="wup", bufs=4))
        wdp = ctx.enter_context(tc.tile_pool(name="wdp", bufs=3))
        tp = ctx.enter_context(tc.tile_pool(name="tp", bufs=2))
        gap = ctx.enter_context(tc.tile_pool(name="gap", bufs=3))
        yp = ctx.enter_context(tc.tile_pool(name="yp", bufs=1))
        pp = ctx.enter_context(tc.tile_pool(name="pp", bufs=8, space="PSUM"))

        # All input activations in TWO DMAs: host packs xT d-chunks side
        # by side as [128, 8*Wp] (64B-aligned slots so the PE moving-
        # operand slices stay aligned); xts[d] are column slices.
        x_all = xpool.tile([128, _ND * Wp], f32r, tag="xall", name="x_all")
        half = _ND // 2 * Wp
        xts = [x_all[:, d * Wp:d * Wp + W] for d in range(_ND)]

        y_acc = [yp.tile([128, W], f32, tag=f"y{d}", name=f"y_acc{d}")
                 for d in range(_ND)]

        # Warmup scratch: dummy matmuls pre-trip the HAM activity window
        # while x / the first weight tiles are still in flight.
        scr_w = xpool.tile([128, 128], f32r, tag="scrw", name="scr_w")
        scr_x = xpool.tile([128, W], f32r, tag="scrx", name="scr_x")
        nc.vector.memset(scr_w[:], 0.0)
        nc.vector.memset(scr_x[:], 0.0)
        scr_p = [pp.tile([128, W], f32, tag="ps", name=f"scr_p{i}")
                 for i in range(2)]
        for i in range(6):
            nc.tensor.matmul(scr_p[i % 2][:], scr_w[:], scr_x[:],
                             start=True, stop=True)

        # Group 0 weight tiles: wg on sync ring, wu on the (idle) scalar
        # ring, in d order so the d-outer opening consumes them as they
        # land.
        nc.sync.dma_start(x_all[:, 0:half], xt[:, 0:half])
        wg0, wu0 = [], []
        for d in range(_ND):
            g_t = wgp.tile([128, _FSG], f32r, tag=f"wg{d}", name=f"wg0_{d}")
            wg0.append(g_t)
            u_t = wup.tile([128, _FSG], f32r, tag=f"wu{d}", name=f"wu0_{d}")
            wu0.append(u_t)
        for d in range(3):
            nc.sync.dma_start(wg0[d][:], wg[0, d])
            nc.scalar.dma_start(wu0[d][:], wu[0, d])
        nc.sync.dma_start(x_all[:, half:], xt[:, half:])
        for d in range(3, _ND):
            nc.sync.dma_start(wg0[d][:], wg[0, d])
            nc.scalar.dma_start(wu0[d][:], wu[0, d])

        def _swiglu(psg, psu, ft):
            g_act = gap.tile([128, W], f32, tag="gact", name=f"ga_{_nm()}")
            nc.scalar.activation(g_act[:], psg[:],
                                 mybir.ActivationFunctionType.Silu)
            t_t = tp.tile([128, W], f32r, tag=f"t{ft}", name=f"t_{_nm()}")
            nc.vector.tensor_mul(t_t[:], g_act[:], psu[:])
            return t_t

        # Down-projection of the PREVIOUS f group is interleaved between
        # this group's matmul bursts.
        def emit_down(fsg, t_tiles, wd_tiles, dts, last=False):
            for dt in dts:
                pdt = pp.tile([128, W], f32, tag="ps", name=f"pd_{_nm()}")
                for ft in range(_FTG):
                    nc.tensor.matmul(
                        pdt[:],
                        wd_tiles[ft][:, dt * 128:(dt + 1) * 128],
                        t_tiles[ft][:],
                        start=(ft == 0), stop=(ft == _FTG - 1))
                if fsg == 0:
                    nc.vector.tensor_copy(y_acc[dt][:], pdt[:])
                else:
                    nc.vector.tensor_add(y_acc[dt][:], y_acc[dt][:], pdt[:])
                if last:
                    eng = nc.sync if dt % 2 == 0 else nc.scalar
                    eng.dma_start(yt[dt * 128:(dt + 1) * 128, :],
                                  y_acc[dt][:])

        # ---------------- group 0 ----------------
        # Phase A: f-tiles 0..3 d-OUTER — chains fill as weight tiles
        # arrive; PE is busy from the first tile instead of waiting for
        # all eight.
        psg0 = [pp.tile([128, W], f32, tag="ps", name=f"pg0_{i}")
                for i in range(4)]
        psu0 = [pp.tile([128, W], f32, tag="ps", name=f"pu0_{i}")
                for i in range(4)]
        for d in range(_ND):
            for ft in range(4):
                nc.tensor.matmul(
                    psg0[ft][:], wg0[d][:, ft * 128:(ft + 1) * 128],
                    xts[d], start=(d == 0), stop=(d == _ND - 1))
                nc.tensor.matmul(
                    psu0[ft][:], wu0[d][:, ft * 128:(ft + 1) * 128],
                    xts[d], start=(d == 0), stop=(d == _ND - 1))
        t_g0 = [_swiglu(psg0[ft], psu0[ft], ft) for ft in range(4)]

        # wd tiles for group 0 (consumed by downs during group 1)
        wd_prev = []
        for ft in range(_FTG):
            wd_t = wdp.tile([128, D_MODEL], f32r, tag=f"wd{ft}",
                            name=f"wd0_{ft}")
            nc.sync.dma_start(wd_t[:], wd[ft * 128:(ft + 1) * 128, :])
            wd_prev.append(wd_t)

        # Phase B: f-tiles 4..7 f-major (all tiles resident by now)
        for ft in range(4, _FTG):
            psg = pp.tile([128, W], f32, tag="ps", name=f"pg_{_nm()}")
            for d in range(_ND):
                nc.tensor.matmul(
                    psg[:], wg0[d][:, ft * 128:(ft + 1) * 128], xts[d],
                    start=(d == 0), stop=(d == _ND - 1))
            psu = pp.tile([128, W], f32, tag="ps", name=f"pu_{_nm()}")
            for d in range(_ND):
                nc.tensor.matmul(
                    psu[:], wu0[d][:, ft * 128:(ft + 1) * 128], xts[d],
                    start=(d == 0), stop=(d == _ND - 1))
            t_g0.append(_swiglu(psg, psu, ft))

        prev = (0, t_g0, wd_prev)

        # ---------------- groups 1..3 ----------------
        for fsg in range(1, _NFSG):
            wg_t, wu_t = [], []
            for d in range(_ND):
                g_t = wgp.tile([128, _FSG], f32r, tag=f"wg{d}",
                               name=f"wg{fsg}_{d}")
                nc.sync.dma_start(g_t[:], wg[fsg, d])
                wg_t.append(g_t)
                u_t = wup.tile([128, _FSG], f32r, tag=f"wu{d}",
                               name=f"wu{fsg}_{d}")
                nc.sync.dma_start(u_t[:], wu[fsg, d])
                wu_t.append(u_t)

            t_tiles = []
            wd_tiles = []
            for ft in range(_FTG):
                fc = fsg * _FTG + ft
                wd_t = wdp.tile([128, D_MODEL], f32r, tag=f"wd{ft}",
                                name=f"wd{fsg}_{ft}")
                nc.sync.dma_start(wd_t[:], wd[fc * 128:(fc + 1) * 128, :])
                wd_tiles.append(wd_t)
                psg = pp.tile([128, W], f32, tag="ps", name=f"pg_{_nm()}")
                for d in range(_ND):
                    nc.tensor.matmul(
                        psg[:], wg_t[d][:, ft * 128:(ft + 1) * 128],
                        xts[d], start=(d == 0), stop=(d == _ND - 1))
                psu = pp.tile([128, W], f32, tag="ps", name=f"pu_{_nm()}")
                for d in range(_ND):
                    nc.tensor.matmul(
                        psu[:], wu_t[d][:, ft * 128:(ft + 1) * 128],
                        xts[d], start=(d == 0), stop=(d == _ND - 1))
                t_tiles.append(_swiglu(psg, psu, ft))
                emit_down(prev[0], prev[1], prev[2], (ft,))
            prev = (fsg, t_tiles, wd_tiles)

        # ---------------- tail: downs of the last group ----------------
        emit_down(prev[0], prev[1], prev[2], range(_ND), last=True)

    nc.compile()
    return nc


def _pack_gu(w):
    # [D, F] -> [NFSG, ND, 128, FSG] so each streamed tile is contiguous
    w = np.asarray(w).astype(_np_cdt())
    return np.ascontiguousarray(
        w.reshape(_ND, 128, _NFSG, _FSG).transpose(2, 0, 1, 3))


def _run_one(W, tok_lists, x_flat, packed_w, out_flat):
    from concourse.bass_utils import run_bass_kernel_spmd

    if W not in _nc_cache:
        _nc_cache[W] = _build_nc(W)
    nc = _nc_cache[W]

    D = x_flat.shape[1]
    in_maps = []
    for e in range(N_EXPERTS):
        toks = tok_lists[e]
        # xT packed as [128, ND*Wp]: d-chunk d at cols [d*Wp, d*Wp+W)
        Wp = (W + 31) // 32 * 32
        xt_e = np.zeros((128, _ND * Wp), dtype=_np_cdt())
        xe = x_flat[toks].T.astype(_np_cdt())          # [D, n]
        n = len(toks)
        for d in range(_ND):
            xt_e[:, d * Wp:d * Wp + n] = xe[d * 128:(d + 1) * 128, :]
        in_maps.append({
            "xt": xt_e,
            "wg": packed_w[e][0],
            "wu": packed_w[e][1],
            "wd": packed_w[e][2],
        })

    res = None
    for attempt in range(3):
        try:
            res = run_bass_kernel_spmd(nc, in_maps,
                                       core_ids=list(range(N_CORES)))
            break
        except Exception:
            if attempt == 2:
                raise
            import time
            time.sleep(3.0)
            try:
                import jax
                jax.clear_caches()
                jax.clear_backends()
            except Exception:
                pass
    for e in range(N_EXPERTS):
        toks = tok_lists[e]
        out_flat[toks] = res.results[e]["yt"][:, :len(toks)].T


def kernel(x, expert_idx, w_gate, w_up, w_down):
    x = np.asarray(x, dtype=np.float32)
    idx = np.asarray(expert_idx).astype(np.int64)
    B, S, D = x.shape
    T = B * S
    x_flat = np.ascontiguousarray(x.reshape(T, D))
    idx_flat = idx.reshape(T)

    packed_w = [
        (_pack_gu(w_gate[e]), _pack_gu(w_up[e]),
         np.ascontiguousarray(np.asarray(w_down[e]).astype(_np_cdt())))
        for e in range(N_EXPERTS)
    ]

    tok_lists = [np.nonzero(idx_flat == e)[0] for e in range(N_EXPERTS)]
    cap = max(1, max(len(t) for t in tok_lists))
    out_flat = np.zeros((T, D), dtype=np.float32)

    if cap <= 512:
        # normal path: one SPMD run, capacity = max expert load (floor 256
        # keeps DMA partition lines >= 512B)
        W = max(256, cap)
        _run_one(W, tok_lists, x_flat, packed_w, out_flat)
    else:
        # fallback for extreme routing imbalance: process tokens in
        # rounds of <=512 per expert, reusing one compiled W=512 program
        rounds = -(-cap // 512)
        for r in range(rounds):
            round_lists = [t[r * 512:(r + 1) * 512] for t in tok_lists]
            _run_one(512, round_lists, x_flat, packed_w, out_flat)

    return out_flat.reshape(B, S, D)
